# revision 34
# baseline (speedup 1.0000x reference)
"""Trainium2 Bass kernel for nn_CONV_A_64115271795341.

The module (im2col mean-centered conv + linear on window means) folds into
a single 3x3 edge-padded conv with host-folded effective weights
  W_eff[c,k,d] = weight[c,k,d] + (w_lin[d,c] - sum_k weight[c,k,d]) / 9.
Matmuls execute serially on the PE queue; wall time ~ total rhs columns.
This kernel packs 3 kernel taps into every matmul (99 total, 3 cols per
output pixel + 384 head cols = 49,536 columns ~ 20.6us at 2.4GHz fp16):

  - SBUF xp[128, NP] fp16: partitions 0-63 = padded image (row-major,
    WP=130), partitions 64-127 = same shifted one row (host-prepped), so
    K=128 contracts two vertically adjacent taps at once.
  - For output span h0..h0+3 (N=512) and kernel col j, ONE matmul with
    rhs base row h0+1 and lhsT[128, 128]:
      cols 0-63  (alpha): rows 0-63 = W(1,j), rows 64-127 = W(2,j)
        -> psum[0:64]  += taps (1,j)+(2,j), aligned with the span.
      cols 64-127 (beta): rows 0-63 = W(0,j), rows 64-127 = 0
        -> psum[64:128] += tap (0,j) partials leading by one output row.
    j=0,1,2 accumulate -> 3 matmuls cover all 9 taps; output row 0's
    beta piece comes from 3 head matmuls (N=128) on x row 0 at rep start.
  - psum groups of SPG=2 spans (2 banks) x 4 pool buffers = all 8 banks.
  - Drain per group: one partition-crossing ACT copy stages the beta
    half psP[64:128] into sbB at W + 1024*g (so beta for output col o
    sits at sbB[:, o]); one DVE add outt[:, o] = psP[0:64, .] + sbB[:, o]
    (DVE tensor_tensor requires equal base partitions when both inputs
    are SBUF, so alpha is read straight from psum).
  - Next iteration's w_sb/xp DMAs are prefetched AND the 16 image chunks
    are staggered one per psum group, so input transfers trickle through
    the iteration instead of bursting against the PE's SBUF operand
    reads (measured ~1us on HW vs a single prefetch burst; one big
    transfer is the worst variant); the PE never stalls at the repeat
    boundary. Output DMA is likewise issued as 16 per-group chunks on
    the SP queue (not the ACT sequencer, which is busy with staging
    copies; not gpsimd, whose SWDGE descriptor generation costs ~1us of
    Pool-engine time per chunk) as soon as each group's add completes.
  - 8 images data-parallel across 8 cores; weights replicated.
"""

import numpy as np

C, H, W, D, B = 64, 128, 128, 64, 8
KS = 3
WP = W + 2            # 130
HP = H + 2
NP = WP * HP          # 16900
TILE_ROWS = 4
TN = TILE_ROWS * W    # 512
NSPANS = H // TILE_ROWS   # 32
SPG = 2                   # spans per psum group
NG = NSPANS // SPG        # 16 groups
G = SPG * TN              # 1024 cols per group

_CACHE = {}
REPLICATE = "host"   # "host": send 2 image copies; "device": 1 copy + s2s


def _build(repeat=1, in_chunks=16, out_chunks=16, psum_bufs=4, xp_bufs=2,
           out_bufs=2, stg_dtype="float16", stg_bufs=2, prefetch_g=0,
           head_g=13, stagger_g=1, out_q="sync", replicate=None,
           skip_in=False, skip_out=False, skip_act=False, skip_dve=False):
    if replicate is None:
        replicate = REPLICATE
    # staggered prefetch must land every chunk within the NG groups
    assert stagger_g == 0 or repeat == 1 or \
        prefetch_g + (in_chunks - 1) * stagger_g <= NG - 1, \
        "staggered chunks would not all be emitted"
    import concourse.bass as bass  # noqa: F401
    import concourse.mybir as mybir
    import concourse.tile as tile
    from concourse import bacc

    dt = mybir.dt
    sdt = getattr(dt, stg_dtype)
    nc = bacc.Bacc("TRN2", target_bir_lowering=False, debug=False, num_devices=8)

    x_parts = 128 if replicate == "host" else 64
    x_d = nc.dram_tensor("x", [x_parts, NP], dt.float16, kind="ExternalInput")
    w_d = nc.dram_tensor("w", [128, KS * 128], dt.float16, kind="ExternalInput")
    out_d = nc.dram_tensor("out", [D, H * W], dt.float16, kind="ExternalOutput")

    with tile.TileContext(nc) as tc:
        with tc.tile_pool(name="io", bufs=xp_bufs) as io_pool, \
             tc.tile_pool(name="wp", bufs=2) as w_pool, \
             tc.tile_pool(name="outp", bufs=out_bufs) as out_pool, \
             tc.tile_pool(name="stg", bufs=stg_bufs) as stg_pool, \
             tc.tile_pool(name="ps", bufs=psum_bufs, space="PSUM") as ps_pool:

            bnd = [NP * g // in_chunks for g in range(in_chunks + 1)]

            def alloc_inputs():
                w_sb = w_pool.tile([128, KS * 128], dt.float16,
                                   name="w_sb", tag="w_sb")
                nc.sync.dma_start(w_sb[:, :], w_d.ap()[:, :])
                xp = io_pool.tile([128, NP], dt.float16, name="xp", tag="xp")
                return w_sb, xp

            def emit_chunk(xp, c):
                a, b = bnd[c], bnd[c + 1]
                if skip_in:
                    if c == 0:
                        nc.sync.dma_start(xp[:, 0:64], x_d.ap()[:, 0:64])
                elif replicate == "host":
                    nc.sync.dma_start(xp[:, a:b], x_d.ap()[:, a:b])
                else:
                    # one HBM copy; row-shifted second copy built
                    # on-device (SBUF->SBUF, no HBM traffic)
                    nc.sync.dma_start(xp[0:64, a:b], x_d.ap()[:, a:b])
                    sa, sb = max(WP, a), b
                    if sb > sa:
                        nc.sync.dma_start(xp[64:128, sa - WP:sb - WP],
                                          xp[0:64, sa:sb])

            def emit_inputs():
                w_sb, xp = alloc_inputs()
                for c in range(in_chunks):
                    emit_chunk(xp, c)
                return w_sb, xp

            nxt = emit_inputs()
            obnd = [H * W * g // out_chunks for g in range(out_chunks + 1)]
            # group index after which output chunk c is complete
            out_after = {}
            for c in range(out_chunks):
                out_after[(obnd[c + 1] + G - 1) // G - 1] = c

            def emit_head(w_sb, xp):
                # head: beta (tap-row-0) partials for output row 0,
                # from x row 0 (3 matmuls, N=128)
                xv = xp.rearrange("p (r c) -> p r c", c=WP)
                psQ = ps_pool.tile([128, G], mybir.dt.float32,
                                   name="psP", tag="psP")
                for j in range(KS):
                    nc.tensor.matmul(
                        psQ[0:64, 0:W],
                        lhsT=w_sb[0:64, 128 * j + 64:128 * j + 128],
                        rhs=xv[0:64, 0:1, j:j + W],
                        start=(j == 0), stop=(j == KS - 1),
                    )
                return psQ

            head_ps = None
            for _rep in range(repeat):
                w_sb, xp = nxt
                xv = xp.rearrange("p (r c) -> p r c", c=WP)
                outt = None if skip_dve else out_pool.tile(
                    [D, H * W], dt.float16, name="outt", tag="outt")
                sbB = stg_pool.tile([64, H * W + W], sdt,
                                    name="sbB", tag="sbB")

                # head matmuls are pipelined: emitted during the previous
                # iteration (at head_g) so the psum-pool rotation never
                # blocks the PE at the repeat boundary.
                psQ = head_ps if head_ps is not None else emit_head(w_sb, xp)
                head_ps = None
                nc.scalar.copy(sbB[:, 0:W], psQ[0:64, 0:W])

                for g in range(NG):
                    psP = ps_pool.tile([128, G], mybir.dt.float32,
                                       name="psP", tag="psP")
                    # j-outer: consecutive matmuls share lhsT
                    for j in range(KS):
                        for s in range(SPG):
                            h0 = TILE_ROWS * (SPG * g + s)
                            nc.tensor.matmul(
                                psP[:, TN * s:TN * (s + 1)],
                                lhsT=w_sb[:, 128 * j:128 * (j + 1)],
                                rhs=xv[:, h0 + 1:h0 + 1 + TILE_ROWS, j:j + W],
                                start=(j == 0), stop=(j == KS - 1),
                            )
                    if _rep + 1 < repeat:
                        if stagger_g == 0:
                            if g == prefetch_g:
                                nxt = emit_inputs()
                        else:
                            # spread prefetch chunks across the iteration so
                            # input transfers trickle instead of bursting
                            # against the PE's SBUF reads
                            if g == prefetch_g:
                                nxt = alloc_inputs()
                                emit_chunk(nxt[1], 0)
                            for c in range(1, in_chunks):
                                if g == prefetch_g + c * stagger_g:
                                    emit_chunk(nxt[1], c)
                    if g == head_g and _rep + 1 < repeat:
                        head_ps = emit_head(*nxt)
                    # stage this group's beta partials contiguously
                    if not skip_act:
                        nc.scalar.copy(sbB[:, W + G * g:W + G * (g + 1)],
                                       psP[64:128, :])
                    if not skip_dve:
                        nc.vector.tensor_add(
                            outt[:, G * g:G * (g + 1)],
                            psP[0:64, :],
                            sbB[:, G * g:G * (g + 1)])
                    if g in out_after and not skip_out:
                        c = out_after[g]
                        a, b = obnd[c], obnd[c + 1]
                        eng = getattr(nc, out_q)
                        eng.dma_start(out_d.ap()[:, a:b], outt[:, a:b])

    nc.compile()
    return nc


def _prep_inputs(x, weight, w_lin):
    w = np.asarray(weight).astype(np.float64)
    weff = w + (np.asarray(w_lin).astype(np.float64).T[:, None, :]
                - w.sum(axis=1, keepdims=True)) / 9.0
    weff = weff.astype(np.float32)                      # [C, 9, D]
    w_sb = np.zeros((128, KS * 128), np.float16)
    for j in range(KS):
        w_sb[0:C, 128 * j:128 * j + 64] = weff[:, 1 * KS + j, :]
        w_sb[C:128, 128 * j:128 * j + 64] = weff[:, 2 * KS + j, :]
        w_sb[0:C, 128 * j + 64:128 * j + 128] = weff[:, 0 * KS + j, :]

    xpad = np.pad(np.asarray(x), ((0, 0), (0, 0), (1, 1), (1, 1)), mode="edge")
    xpad = xpad.reshape(B, C, NP).astype(np.float16)
    if REPLICATE == "device":
        return xpad, w_sb
    xh = np.zeros((B, 128, NP), np.float16)
    xh[:, 0:C, :] = xpad
    xh[:, C:128, 0:NP - WP] = xpad[:, :, WP:]
    return xh, w_sb


def kernel(x, weight, w_lin):
    from concourse.bass_utils import run_bass_kernel_spmd

    if "nc" not in _CACHE:
        _CACHE["nc"] = _build()
    nc = _CACHE["nc"]

    xh, w_sb = _prep_inputs(x, weight, w_lin)
    in_maps = [{"x": xh[b], "w": w_sb} for b in range(B)]
    res = run_bass_kernel_spmd(nc, in_maps, core_ids=list(range(B)))
    out = np.stack([res.results[b]["out"].reshape(D, H, W) for b in range(B)])
    return out.astype(np.float32)


# revision 36
# speedup vs baseline: 1.0054x; 1.0054x over previous
"""Trainium2 Bass kernel for nn_CONV_A_64115271795341.

The module (im2col mean-centered conv + linear on window means) folds into
a single 3x3 edge-padded conv with host-folded effective weights
  W_eff[c,k,d] = weight[c,k,d] + (w_lin[d,c] - sum_k weight[c,k,d]) / 9.
Matmuls execute serially on the PE queue; wall time ~ total rhs columns.
This kernel packs 3 kernel taps into every matmul (99 total, 3 cols per
output pixel + 384 head cols = 49,536 columns ~ 20.6us at 2.4GHz fp16):

  - SBUF xp[128, NP] fp16: partitions 0-63 = padded image (row-major,
    WP=130), partitions 64-127 = same shifted one row (host-prepped), so
    K=128 contracts two vertically adjacent taps at once.
  - For output span h0..h0+3 (N=512) and kernel col j, ONE matmul with
    rhs base row h0+1 and lhsT[128, 128]:
      cols 0-63  (alpha): rows 0-63 = W(1,j), rows 64-127 = W(2,j)
        -> psum[0:64]  += taps (1,j)+(2,j), aligned with the span.
      cols 64-127 (beta): rows 0-63 = W(0,j), rows 64-127 = 0
        -> psum[64:128] += tap (0,j) partials leading by one output row.
    j=0,1,2 accumulate -> 3 matmuls cover all 9 taps; output row 0's
    beta piece comes from 3 head matmuls (N=128) on x row 0 at rep start.
  - psum groups of SPG=2 spans (2 banks) x 4 pool buffers = all 8 banks.
  - Drain per group: one partition-crossing ACT copy stages the beta
    half psP[64:128] into sbB at W + 1024*g (so beta for output col o
    sits at sbB[:, o]); one DVE add outt[:, o] = psP[0:64, .] + sbB[:, o]
    (DVE tensor_tensor requires equal base partitions when both inputs
    are SBUF, so alpha is read straight from psum).
  - Next iteration's w_sb/xp DMAs are prefetched AND the 16 image chunks
    are staggered one per psum group, so input transfers trickle through
    the iteration instead of bursting against the PE's SBUF operand
    reads (measured ~1us on HW vs a single prefetch burst; one big
    transfer is the worst variant); the PE never stalls at the repeat
    boundary. Output DMA is likewise issued as 16 per-group chunks on
    the SP queue (not the ACT sequencer, which is busy with staging
    copies; not gpsimd, whose SWDGE descriptor generation costs ~1us of
    Pool-engine time per chunk) as soon as each group's add completes.
  - 8 images data-parallel across 8 cores; weights replicated.
"""

import numpy as np

C, H, W, D, B = 64, 128, 128, 64, 8
KS = 3
WP = W + 2            # 130
HP = H + 2
NP = WP * HP          # 16900
TILE_ROWS = 4
TN = TILE_ROWS * W    # 512
NSPANS = H // TILE_ROWS   # 32
SPG = 2                   # spans per psum group
NG = NSPANS // SPG        # 16 groups
G = SPG * TN              # 1024 cols per group

_CACHE = {}
REPLICATE = "host"   # "host": send 2 image copies; "device": 1 copy + s2s


def _build(repeat=1, in_chunks=16, out_chunks=16, psum_bufs=4, xp_bufs=2,
           out_bufs=2, stg_dtype="float16", stg_bufs=2, prefetch_g=0,
           head_g=13, stagger_g=1, out_q="sync", replicate=None,
           skip_in=False, skip_out=False, skip_act=False, skip_dve=False):
    if replicate is None:
        replicate = REPLICATE
    # staggered prefetch must land every chunk within the NG groups
    _span = (NG - prefetch_g) * stagger_g
    assert stagger_g == 0 or repeat == 1 or \
        prefetch_g + ((in_chunks - 1) * _span) // in_chunks <= NG - 1, \
        "staggered chunks would not all be emitted"
    import concourse.bass as bass  # noqa: F401
    import concourse.mybir as mybir
    import concourse.tile as tile
    from concourse import bacc

    dt = mybir.dt
    sdt = getattr(dt, stg_dtype)
    nc = bacc.Bacc("TRN2", target_bir_lowering=False, debug=False, num_devices=8)

    x_parts = 128 if replicate == "host" else 64
    x_d = nc.dram_tensor("x", [x_parts, NP], dt.float16, kind="ExternalInput")
    w_d = nc.dram_tensor("w", [128, KS * 128], dt.float16, kind="ExternalInput")
    out_d = nc.dram_tensor("out", [D, H * W], dt.float16, kind="ExternalOutput")

    with tile.TileContext(nc) as tc:
        with tc.tile_pool(name="io", bufs=xp_bufs) as io_pool, \
             tc.tile_pool(name="wp", bufs=2) as w_pool, \
             tc.tile_pool(name="outp", bufs=out_bufs) as out_pool, \
             tc.tile_pool(name="stg", bufs=stg_bufs) as stg_pool, \
             tc.tile_pool(name="ps", bufs=psum_bufs, space="PSUM") as ps_pool:

            bnd = [NP * g // in_chunks for g in range(in_chunks + 1)]

            def alloc_inputs():
                w_sb = w_pool.tile([128, KS * 128], dt.float16,
                                   name="w_sb", tag="w_sb")
                nc.sync.dma_start(w_sb[:, :], w_d.ap()[:, :])
                xp = io_pool.tile([128, NP], dt.float16, name="xp", tag="xp")
                return w_sb, xp

            def emit_chunk(xp, c):
                a, b = bnd[c], bnd[c + 1]
                if skip_in:
                    if c == 0:
                        nc.sync.dma_start(xp[:, 0:64], x_d.ap()[:, 0:64])
                elif replicate == "host":
                    nc.sync.dma_start(xp[:, a:b], x_d.ap()[:, a:b])
                else:
                    # one HBM copy; row-shifted second copy built
                    # on-device (SBUF->SBUF, no HBM traffic)
                    nc.sync.dma_start(xp[0:64, a:b], x_d.ap()[:, a:b])
                    sa, sb = max(WP, a), b
                    if sb > sa:
                        nc.sync.dma_start(xp[64:128, sa - WP:sb - WP],
                                          xp[0:64, sa:sb])

            def emit_inputs():
                w_sb, xp = alloc_inputs()
                for c in range(in_chunks):
                    emit_chunk(xp, c)
                return w_sb, xp

            nxt = emit_inputs()
            obnd = [H * W * g // out_chunks for g in range(out_chunks + 1)]
            # group index after which output chunk c is complete
            out_after = {}
            for c in range(out_chunks):
                out_after[(obnd[c + 1] + G - 1) // G - 1] = c

            def emit_head(w_sb, xp):
                # head: beta (tap-row-0) partials for output row 0,
                # from x row 0 (3 matmuls, N=128)
                xv = xp.rearrange("p (r c) -> p r c", c=WP)
                psQ = ps_pool.tile([128, G], mybir.dt.float32,
                                   name="psP", tag="psP")
                for j in range(KS):
                    nc.tensor.matmul(
                        psQ[0:64, 0:W],
                        lhsT=w_sb[0:64, 128 * j + 64:128 * j + 128],
                        rhs=xv[0:64, 0:1, j:j + W],
                        start=(j == 0), stop=(j == KS - 1),
                    )
                return psQ

            head_ps = None
            for _rep in range(repeat):
                w_sb, xp = nxt
                xv = xp.rearrange("p (r c) -> p r c", c=WP)
                outt = None if skip_dve else out_pool.tile(
                    [D, H * W], dt.float16, name="outt", tag="outt")
                sbB = stg_pool.tile([64, H * W + W], sdt,
                                    name="sbB", tag="sbB")

                # head matmuls are pipelined: emitted during the previous
                # iteration (at head_g) so the psum-pool rotation never
                # blocks the PE at the repeat boundary.
                psQ = head_ps if head_ps is not None else emit_head(w_sb, xp)
                head_ps = None
                nc.scalar.copy(sbB[:, 0:W], psQ[0:64, 0:W])

                for g in range(NG):
                    psP = ps_pool.tile([128, G], mybir.dt.float32,
                                       name="psP", tag="psP")
                    # j-outer: consecutive matmuls share lhsT
                    for j in range(KS):
                        for s in range(SPG):
                            h0 = TILE_ROWS * (SPG * g + s)
                            nc.tensor.matmul(
                                psP[:, TN * s:TN * (s + 1)],
                                lhsT=w_sb[:, 128 * j:128 * (j + 1)],
                                rhs=xv[:, h0 + 1:h0 + 1 + TILE_ROWS, j:j + W],
                                start=(j == 0), stop=(j == KS - 1),
                            )
                    if _rep + 1 < repeat:
                        if stagger_g == 0:
                            if g == prefetch_g:
                                nxt = emit_inputs()
                        else:
                            # spread prefetch chunks across the iteration so
                            # input transfers trickle instead of bursting
                            # against the PE's SBUF reads; >NG chunks emit
                            # several smaller ones per group
                            if g == prefetch_g:
                                nxt = alloc_inputs()
                            span = (NG - prefetch_g) * stagger_g
                            for c in range(in_chunks):
                                if g == prefetch_g + (c * span) // in_chunks:
                                    emit_chunk(nxt[1], c)
                    if g == head_g and _rep + 1 < repeat:
                        head_ps = emit_head(*nxt)
                    # stage this group's beta partials contiguously
                    if not skip_act:
                        nc.scalar.copy(sbB[:, W + G * g:W + G * (g + 1)],
                                       psP[64:128, :])
                    if not skip_dve:
                        nc.vector.tensor_add(
                            outt[:, G * g:G * (g + 1)],
                            psP[0:64, :],
                            sbB[:, G * g:G * (g + 1)])
                    if g in out_after and not skip_out:
                        c = out_after[g]
                        a, b = obnd[c], obnd[c + 1]
                        eng = getattr(nc, out_q)
                        eng.dma_start(out_d.ap()[:, a:b], outt[:, a:b])

    nc.compile()
    return nc


def _prep_inputs(x, weight, w_lin):
    w = np.asarray(weight).astype(np.float64)
    weff = w + (np.asarray(w_lin).astype(np.float64).T[:, None, :]
                - w.sum(axis=1, keepdims=True)) / 9.0
    weff = weff.astype(np.float32)                      # [C, 9, D]
    w_sb = np.zeros((128, KS * 128), np.float16)
    for j in range(KS):
        w_sb[0:C, 128 * j:128 * j + 64] = weff[:, 1 * KS + j, :]
        w_sb[C:128, 128 * j:128 * j + 64] = weff[:, 2 * KS + j, :]
        w_sb[0:C, 128 * j + 64:128 * j + 128] = weff[:, 0 * KS + j, :]

    xpad = np.pad(np.asarray(x), ((0, 0), (0, 0), (1, 1), (1, 1)), mode="edge")
    xpad = xpad.reshape(B, C, NP).astype(np.float16)
    if REPLICATE == "device":
        return xpad, w_sb
    xh = np.zeros((B, 128, NP), np.float16)
    xh[:, 0:C, :] = xpad
    xh[:, C:128, 0:NP - WP] = xpad[:, :, WP:]
    return xh, w_sb


def kernel(x, weight, w_lin):
    from concourse.bass_utils import run_bass_kernel_spmd

    if "nc" not in _CACHE:
        _CACHE["nc"] = _build()
    nc = _CACHE["nc"]

    xh, w_sb = _prep_inputs(x, weight, w_lin)
    in_maps = [{"x": xh[b], "w": w_sb} for b in range(B)]
    res = run_bass_kernel_spmd(nc, in_maps, core_ids=list(range(B)))
    out = np.stack([res.results[b]["out"].reshape(D, H, W) for b in range(B)])
    return out.astype(np.float32)


# revision 39
# speedup vs baseline: 1.0138x; 1.0083x over previous
"""Trainium2 Bass kernel for nn_CONV_A_64115271795341.

The module (im2col mean-centered conv + linear on window means) folds into
a single 3x3 edge-padded conv with host-folded effective weights
  W_eff[c,k,d] = weight[c,k,d] + (w_lin[d,c] - sum_k weight[c,k,d]) / 9.
Matmuls execute serially on the PE queue; wall time ~ total rhs columns.
This kernel packs 3 kernel taps into every matmul (99 total, 3 cols per
output pixel + 384 head cols = 49,536 columns ~ 20.6us at 2.4GHz fp16):

  - SBUF xp[128, NP] fp16: partitions 0-63 = padded image (row-major,
    WP=130), partitions 64-127 = same shifted one row (host-prepped), so
    K=128 contracts two vertically adjacent taps at once.
  - For output span h0..h0+3 (N=512) and kernel col j, ONE matmul with
    rhs base row h0+1 and lhsT[128, 128]:
      cols 0-63  (alpha): rows 0-63 = W(1,j), rows 64-127 = W(2,j)
        -> psum[0:64]  += taps (1,j)+(2,j), aligned with the span.
      cols 64-127 (beta): rows 0-63 = W(0,j), rows 64-127 = 0
        -> psum[64:128] += tap (0,j) partials leading by one output row.
    j=0,1,2 accumulate -> 3 matmuls cover all 9 taps; output row 0's
    beta piece comes from 3 head matmuls (N=128) on x row 0 at rep start.
  - psum groups of SPG=2 spans (2 banks) x 4 pool buffers = all 8 banks.
  - Drain per group: one partition-crossing ACT copy stages the beta
    half psP[64:128] into sbB at W + 1024*g (so beta for output col o
    sits at sbB[:, o]); one DVE add outt[:, o] = psP[0:64, .] + sbB[:, o]
    (DVE tensor_tensor requires equal base partitions when both inputs
    are SBUF, so alpha is read straight from psum).
  - Next iteration's w_sb/xp DMAs are prefetched AND the 16 image chunks
    are staggered one per psum group, so input transfers trickle through
    the iteration instead of bursting against the PE's SBUF operand
    reads (measured ~1us on HW vs a single prefetch burst; one big
    transfer is the worst variant); the PE never stalls at the repeat
    boundary. Output DMA is likewise issued as 16 per-group chunks on
    the SP queue (not the ACT sequencer, which is busy with staging
    copies; not gpsimd, whose SWDGE descriptor generation costs ~1us of
    Pool-engine time per chunk) as soon as each group's add completes.
  - 8 images data-parallel across 8 cores; weights replicated.
"""

import numpy as np

C, H, W, D, B = 64, 128, 128, 64, 8
KS = 3
WP = W + 2            # 130
HP = H + 2
NP = WP * HP          # 16900
TILE_ROWS = 4
TN = TILE_ROWS * W    # 512
NSPANS = H // TILE_ROWS   # 32
SPG = 2                   # spans per psum group
NG = NSPANS // SPG        # 16 groups
G = SPG * TN              # 1024 cols per group

_CACHE = {}
REPLICATE = "host"   # "host": send 2 image copies; "device": 1 copy + s2s


def _build(repeat=1, in_chunks=16, out_chunks=16, psum_bufs=4, xp_bufs=2,
           out_bufs=2, stg_dtype="float16", stg_bufs=2, prefetch_g=0,
           head_g=13, stagger_g=1, out_q="sync", in_q="gpsimd",
           replicate=None,
           skip_in=False, skip_out=False, skip_act=False, skip_dve=False):
    if replicate is None:
        replicate = REPLICATE
    # staggered prefetch must land every chunk within the NG groups
    _span = (NG - prefetch_g) * stagger_g
    assert stagger_g == 0 or repeat == 1 or \
        prefetch_g + ((in_chunks - 1) * _span) // in_chunks <= NG - 1, \
        "staggered chunks would not all be emitted"
    import concourse.bass as bass  # noqa: F401
    import concourse.mybir as mybir
    import concourse.tile as tile
    from concourse import bacc

    dt = mybir.dt
    sdt = getattr(dt, stg_dtype)
    nc = bacc.Bacc("TRN2", target_bir_lowering=False, debug=False, num_devices=8)

    x_parts = 128 if replicate == "host" else 64
    x_d = nc.dram_tensor("x", [x_parts, NP], dt.float16, kind="ExternalInput")
    w_d = nc.dram_tensor("w", [128, KS * 128], dt.float16, kind="ExternalInput")
    out_d = nc.dram_tensor("out", [D, H * W], dt.float16, kind="ExternalOutput")

    with tile.TileContext(nc) as tc:
        with tc.tile_pool(name="io", bufs=xp_bufs) as io_pool, \
             tc.tile_pool(name="wp", bufs=2) as w_pool, \
             tc.tile_pool(name="outp", bufs=out_bufs) as out_pool, \
             tc.tile_pool(name="stg", bufs=stg_bufs) as stg_pool, \
             tc.tile_pool(name="ps", bufs=psum_bufs, space="PSUM") as ps_pool:

            bnd = [NP * g // in_chunks for g in range(in_chunks + 1)]

            def alloc_inputs():
                w_sb = w_pool.tile([128, KS * 128], dt.float16,
                                   name="w_sb", tag="w_sb")
                nc.sync.dma_start(w_sb[:, :], w_d.ap()[:, :])
                xp = io_pool.tile([128, NP], dt.float16, name="xp", tag="xp")
                return w_sb, xp

            def emit_chunk(xp, c):
                a, b = bnd[c], bnd[c + 1]
                ie = getattr(nc, in_q)
                if skip_in:
                    if c == 0:
                        ie.dma_start(xp[:, 0:64], x_d.ap()[:, 0:64])
                elif replicate == "host":
                    ie.dma_start(xp[:, a:b], x_d.ap()[:, a:b])
                else:
                    # one HBM copy; row-shifted second copy built
                    # on-device (SBUF->SBUF, no HBM traffic)
                    ie.dma_start(xp[0:64, a:b], x_d.ap()[:, a:b])
                    sa, sb = max(WP, a), b
                    if sb > sa:
                        ie.dma_start(xp[64:128, sa - WP:sb - WP],
                                     xp[0:64, sa:sb])

            def emit_inputs():
                w_sb, xp = alloc_inputs()
                for c in range(in_chunks):
                    emit_chunk(xp, c)
                return w_sb, xp

            nxt = emit_inputs()
            obnd = [H * W * g // out_chunks for g in range(out_chunks + 1)]
            # group index after which output chunk c is complete
            out_after = {}
            for c in range(out_chunks):
                out_after[(obnd[c + 1] + G - 1) // G - 1] = c

            def emit_head(w_sb, xp):
                # head: beta (tap-row-0) partials for output row 0,
                # from x row 0 (3 matmuls, N=128)
                xv = xp.rearrange("p (r c) -> p r c", c=WP)
                psQ = ps_pool.tile([128, G], mybir.dt.float32,
                                   name="psP", tag="psP")
                for j in range(KS):
                    nc.tensor.matmul(
                        psQ[0:64, 0:W],
                        lhsT=w_sb[0:64, 128 * j + 64:128 * j + 128],
                        rhs=xv[0:64, 0:1, j:j + W],
                        start=(j == 0), stop=(j == KS - 1),
                    )
                return psQ

            head_ps = None
            for _rep in range(repeat):
                w_sb, xp = nxt
                xv = xp.rearrange("p (r c) -> p r c", c=WP)
                outt = None if skip_dve else out_pool.tile(
                    [D, H * W], dt.float16, name="outt", tag="outt")
                sbB = stg_pool.tile([64, H * W + W], sdt,
                                    name="sbB", tag="sbB")

                # head matmuls are pipelined: emitted during the previous
                # iteration (at head_g) so the psum-pool rotation never
                # blocks the PE at the repeat boundary.
                psQ = head_ps if head_ps is not None else emit_head(w_sb, xp)
                head_ps = None
                nc.scalar.copy(sbB[:, 0:W], psQ[0:64, 0:W])

                for g in range(NG):
                    psP = ps_pool.tile([128, G], mybir.dt.float32,
                                       name="psP", tag="psP")
                    # j-outer: consecutive matmuls share lhsT
                    for j in range(KS):
                        for s in range(SPG):
                            h0 = TILE_ROWS * (SPG * g + s)
                            nc.tensor.matmul(
                                psP[:, TN * s:TN * (s + 1)],
                                lhsT=w_sb[:, 128 * j:128 * (j + 1)],
                                rhs=xv[:, h0 + 1:h0 + 1 + TILE_ROWS, j:j + W],
                                start=(j == 0), stop=(j == KS - 1),
                            )
                    if _rep + 1 < repeat:
                        if stagger_g == 0:
                            if g == prefetch_g:
                                nxt = emit_inputs()
                        else:
                            # spread prefetch chunks across the iteration so
                            # input transfers trickle instead of bursting
                            # against the PE's SBUF reads; >NG chunks emit
                            # several smaller ones per group
                            if g == prefetch_g:
                                nxt = alloc_inputs()
                            span = (NG - prefetch_g) * stagger_g
                            for c in range(in_chunks):
                                if g == prefetch_g + (c * span) // in_chunks:
                                    emit_chunk(nxt[1], c)
                    if g == head_g and _rep + 1 < repeat:
                        head_ps = emit_head(*nxt)
                    # stage this group's beta partials contiguously
                    if not skip_act:
                        nc.scalar.copy(sbB[:, W + G * g:W + G * (g + 1)],
                                       psP[64:128, :])
                    if not skip_dve:
                        nc.vector.tensor_add(
                            outt[:, G * g:G * (g + 1)],
                            psP[0:64, :],
                            sbB[:, G * g:G * (g + 1)])
                    if g in out_after and not skip_out:
                        c = out_after[g]
                        a, b = obnd[c], obnd[c + 1]
                        eng = getattr(nc, out_q)
                        eng.dma_start(out_d.ap()[:, a:b], outt[:, a:b])

    nc.compile()
    return nc


def _prep_inputs(x, weight, w_lin):
    w = np.asarray(weight).astype(np.float64)
    weff = w + (np.asarray(w_lin).astype(np.float64).T[:, None, :]
                - w.sum(axis=1, keepdims=True)) / 9.0
    weff = weff.astype(np.float32)                      # [C, 9, D]
    w_sb = np.zeros((128, KS * 128), np.float16)
    for j in range(KS):
        w_sb[0:C, 128 * j:128 * j + 64] = weff[:, 1 * KS + j, :]
        w_sb[C:128, 128 * j:128 * j + 64] = weff[:, 2 * KS + j, :]
        w_sb[0:C, 128 * j + 64:128 * j + 128] = weff[:, 0 * KS + j, :]

    xpad = np.pad(np.asarray(x), ((0, 0), (0, 0), (1, 1), (1, 1)), mode="edge")
    xpad = xpad.reshape(B, C, NP).astype(np.float16)
    if REPLICATE == "device":
        return xpad, w_sb
    xh = np.zeros((B, 128, NP), np.float16)
    xh[:, 0:C, :] = xpad
    xh[:, C:128, 0:NP - WP] = xpad[:, :, WP:]
    return xh, w_sb


def kernel(x, weight, w_lin):
    from concourse.bass_utils import run_bass_kernel_spmd

    if "nc" not in _CACHE:
        _CACHE["nc"] = _build()
    nc = _CACHE["nc"]

    xh, w_sb = _prep_inputs(x, weight, w_lin)
    in_maps = [{"x": xh[b], "w": w_sb} for b in range(B)]
    res = run_bass_kernel_spmd(nc, in_maps, core_ids=list(range(B)))
    out = np.stack([res.results[b]["out"].reshape(D, H, W) for b in range(B)])
    return out.astype(np.float32)
